# revision 59
# baseline (speedup 1.0000x reference)
"""Trainium2 Bass kernel for nn_Attention_52063593562622 (sparse_attention).

B, H, S, D = 16, 4, 1024, 128.  Data-parallel over the batch dim across the
8 NeuronCores (2 batches per core); attention is replicated, no collectives.
Outputs (out, p_attn) match the jax reference to ~5e-4 scale-relative absmax.

Per (b, h) pipeline, fully software-pipelined across heads/batches by Tile:
    S  = Q K^T                    float32r matmuls (1 cyc/row, ~TF32 rounding)
    u  = S * (1/sqrt(D)) * tm     DVE scalar_tensor_tensor, tm = time*mask
    E' = exp(u)      (fp16)       ScalarE; masked entries -> exp(0) = 1
    Em = E' - cm     (fp16)       DVE STT, cm = 1-mask; fused rowsum -> Z
    p_attn = Em * (1/Z)           GPSIMD tensor_scalar, f32, 2 MiB stores
    out = (Em^T @ V) * (1/Z)      PE fp16 transposes + fp16 matmuls, f32 acc

Key points:
  * tm = time*mask + the (1-mask) subtraction give exactly-0 masked
    probabilities (matching exp(-1e9-max) == 0) with no separate mask pass,
    and handle the time==0 corner cases exactly.
  * Q^T/K^T via PE fp32 transposes, evacuated as float32r (operands must be
    rounded to f32r for the fast matmul path).
  * Em^T via PE fp16 transposes into one PSUM bank per q-tile, evacuated by
    ScalarE; V is DMA-cast to fp16 by SWDGE.
  * Work is spread so no engine exceeds the ~186 us/core DMA floor
    (64 MiB HBM traffic): DVE ~170, ACT ~161, Pool ~135, PE ~100 us; the
    p_attn scaling alternates GPSIMD/DVE on the first/last heads where the
    store stream would otherwise starve.
  * Cost-model (TimelineSim) estimate: ~211.6 us/core, ~88% DMA occupancy.
  * _split_multiwait works around this walrus build's 1-sync-wait limit on
    Drain/DMA instructions by hoisting extra waits onto single-wait nops.
"""

import math
import os
import sys
from contextlib import ExitStack

if "/opt/trn_rl_repo" not in sys.path:
    sys.path.insert(0, "/opt/trn_rl_repo")

# The kernel dispatches through the axon PJRT backend (this container has no
# local /dev/neuron*).  A JAX_PLATFORMS=cpu pin (as some harnesses set for the
# jax reference) would hide the NeuronCores, so widen it before jax loads.
_jp = os.environ.get("JAX_PLATFORMS")
if _jp is not None and _jp != "" and "axon" not in _jp:
    os.environ["JAX_PLATFORMS"] = ""

import numpy as np

import concourse.bass as bass
import concourse.mybir as mybir
from concourse.bass_utils import run_bass_kernel_spmd
from concourse.masks import make_identity
from concourse.tile import TileContext

F32 = mybir.dt.float32
F32R = mybir.dt.float32r
F16 = mybir.dt.float16
I32 = mybir.dt.int32
AF = mybir.ActivationFunctionType
OP = mybir.AluOpType

N_CORES = 8
B, H, S, D = 16, 4, 1024, 128
B_LOC = B // N_CORES
P = 128


def _split_multiwait(nc, max_waits=1):
    """Workaround for this walrus build: Drain/CTRL and DMA pseudo
    instructions only accept ONE sync-wait command.  Hoist extra waits onto
    dedicated single-wait nops on the same engine, placed just before the
    instruction (same sequencer + program order => semantics preserved)."""
    n_split = 0
    for f in nc.m.functions:
        for blk in f.blocks:
            insts = blk.instructions
            if not any(
                inst.sync_info is not None
                and len(inst.sync_info.on_wait) > max_waits
                and type(inst).__name__ != "InstAllEngineBarrier"
                for inst in insts
            ):
                continue
            new_insts = []
            for inst in insts:
                si = inst.sync_info
                waits = list(si.on_wait) if si is not None else []
                if (
                    len(waits) > max_waits
                    and type(inst).__name__ != "InstAllEngineBarrier"
                ):
                    for w in waits[:-max_waits]:
                        nop = mybir.InstNoOp(
                            name=nc.get_next_instruction_name(), ins=[], outs=[]
                        )
                        nop.engine = inst.engine
                        nop.bass_nofuse = True
                        nop.sync_info = mybir.SyncInfo(on_wait=[w], on_update=[])
                        nc.register_instruction(nop)
                        new_insts.append(nop)
                        n_split += 1
                    inst.sync_info = mybir.SyncInfo(
                        on_wait=waits[-max_waits:], on_update=list(si.on_update)
                    )
                new_insts.append(inst)
            blk.instructions = new_insts
    return n_split


def build_attention_nc(B_loc=B_LOC):
    QT_TILES = S // P      # q tiles per (b, h)
    KC = S // P            # contraction chunks for PV
    NK = S                 # score row length
    scale = 1.0 / math.sqrt(D)

    nc = bass.Bass()
    q = nc.dram_tensor("q", [B_loc, H, S, D], F32, kind="ExternalInput")
    k = nc.dram_tensor("k", [B_loc, H, S, D], F32, kind="ExternalInput")
    v = nc.dram_tensor("v", [B_loc, H, S, D], F32, kind="ExternalInput")
    time_t = nc.dram_tensor("time", [B_loc, S, S], F32, kind="ExternalInput")
    mask = nc.dram_tensor("mask", [B_loc, 1, S, S], I32, kind="ExternalInput")
    out = nc.dram_tensor("out", [B_loc, H, S, D], F32, kind="ExternalOutput")
    p_attn = nc.dram_tensor("p_attn", [B_loc, H, S, S], F32, kind="ExternalOutput")

    with TileContext(nc) as tc, ExitStack() as ctx:
        const_p = ctx.enter_context(tc.tile_pool(name="const", bufs=1))
        stage_p = ctx.enter_context(tc.tile_pool(name="stage", bufs=2))
        tmc_p = ctx.enter_context(tc.tile_pool(name="tmc", bufs=QT_TILES + 3))
        cmc_p = ctx.enter_context(tc.tile_pool(name="cmc", bufs=QT_TILES + 3))
        bh_p = ctx.enter_context(tc.tile_pool(name="bh", bufs=2))
        qk_p = ctx.enter_context(tc.tile_pool(name="qk", bufs=3))
        work_p = ctx.enter_context(tc.tile_pool(name="work", bufs=3))
        em_p = ctx.enter_context(tc.tile_pool(name="em", bufs=6))
        z_p = ctx.enter_context(tc.tile_pool(name="z", bufs=8))
        emt_p = ctx.enter_context(tc.tile_pool(name="emt", bufs=3))
        out_p = ctx.enter_context(tc.tile_pool(name="outp", bufs=2))
        pst_p = ctx.enter_context(tc.tile_pool(name="pst", bufs=3))
        # PSUM: 8 banks total = 3 (scores) + 2 (EmT) + 1 (QK evac) + 2 (PV out)
        psum_s = ctx.enter_context(tc.tile_pool(name="psum_s", bufs=2, space="PSUM"))
        psum_b = ctx.enter_context(tc.tile_pool(name="psum_b", bufs=2, space="PSUM"))
        psum_qk = ctx.enter_context(tc.tile_pool(name="psum_qk", bufs=1, space="PSUM"))
        psum_o = ctx.enter_context(tc.tile_pool(name="psum_o", bufs=1, space="PSUM"))

        ident32 = const_p.tile([P, P], F32)
        make_identity(nc, ident32[:])
        ident16 = const_p.tile([P, P], F16)
        make_identity(nc, ident16[:])

        def prep_qk(b, h, act_only=False):
            q_nat = qk_p.tile([P, QT_TILES, D], F32, tag="qk_nat")
            nc.sync.dma_start(
                q_nat[:], q[b, h].rearrange("(so sp) d -> sp so d", sp=P)
            )
            k_nat = qk_p.tile([P, QT_TILES, D], F32, tag="qk_nat")
            nc.sync.dma_start(
                k_nat[:], k[b, h].rearrange("(so sp) d -> sp so d", sp=P)
            )
            qt = bh_p.tile([P, S], F32R, tag="qt")
            kt = bh_p.tile([P, S], F32R, tag="kt")
            for si, (src, dst) in enumerate(((q_nat, qt), (k_nat, kt))):
                for gi, j0 in enumerate(range(0, QT_TILES, 4)):
                    pb = psum_qk.tile([P, 512], F32, tag="qk_evac")
                    for jj in range(4):
                        nc.tensor.transpose(
                            pb[:, jj * P : (jj + 1) * P],
                            src[:, j0 + jj],
                            ident32[:],
                        )
                    if act_only or (si * 2 + gi) % 2 == 0:
                        nc.scalar.copy(dst[:, j0 * P : (j0 + 4) * P], pb[:])
                    else:
                        nc.vector.tensor_copy(dst[:, j0 * P : (j0 + 4) * P], pb[:])
            v16 = bh_p.tile([P, KC, D], F16, tag="v16")
            nc.gpsimd.dma_start(
                v16[:], v[b, h].rearrange("(ko kp) d -> kp ko d", kp=P)
            )
            return qt, kt, v16


        def prep_batch(b, interleave=None):
            # per-batch: tm = time*mask (f32), cm = 1-mask (fp16), chunked
            t_r = time_t[b].rearrange("(qo qp) k -> qp qo k", qp=P)
            m_r = mask[b, 0].rearrange("(qo qp) k -> qp qo k", qp=P)
            tm_tiles, cm_tiles = [], []
            for j0 in range(0, QT_TILES, 2):
                if j0 == 6 and interleave is not None:
                    interleave()
                t_sb = stage_p.tile([P, 2, NK], F32, tag="t_raw")
                nc.sync.dma_start(t_sb[:], t_r[:, j0 : j0 + 2])
                m_sb = stage_p.tile([P, 2, NK], I32, tag="m_raw")
                nc.sync.dma_start(m_sb[:], m_r[:, j0 : j0 + 2])
                eng = nc.gpsimd
                for jj in range(2):
                    tmj = tmc_p.tile([P, NK], F32, tag="tm")
                    eng.tensor_tensor(
                        tmj[:], t_sb[:, jj], m_sb[:, jj], OP.mult
                    )
                    cmj = cmc_p.tile([P, NK], F16, tag="cm")
                    eng.tensor_scalar(
                        cmj[:], m_sb[:, jj], -1.0, 1.0, OP.mult, OP.add
                    )
                    tm_tiles.append(tmj)
                    cm_tiles.append(cmj)
            return tm_tiles, cm_tiles

        qk_pending = []
        prepped = {
            0: prep_batch(0, interleave=lambda: qk_pending.append(prep_qk(0, 0)))
        }

        for b in range(B_loc):
            tm_tiles, cm_tiles = prepped.pop(b)
            for h in range(H):
                qt, kt, v16 = qk_pending.pop(0)
                nxt = (b, h + 1) if h + 1 < H else (
                    (b + 1, 0) if b + 1 < B_loc else None)
                if h == H - 1 and b + 1 < B_loc:
                    prepped[b + 1] = prep_batch(
                        b + 1,
                        interleave=lambda: qk_pending.append(prep_qk(*nxt)),
                    )
                elif nxt is not None:
                    qk_pending.append(prep_qk(*nxt))

                o_sb = out_p.tile([P, QT_TILES, D], F32, tag="o")
                for qt_i in range(QT_TILES):
                    u = work_p.tile([P, NK], F32, tag="u")
                    s_ps = psum_s.tile([P, NK], F32, tag="s")
                    for n0 in range(0, NK, 512):
                        nc.tensor.matmul(
                            s_ps[:, n0 : n0 + 512],
                            qt[:, qt_i * P : (qt_i + 1) * P],
                            kt[:, n0 : n0 + 512],
                            start=True,
                            stop=True,
                        )
                    nc.vector.scalar_tensor_tensor(
                        u[:],
                        s_ps[:],
                        scale,
                        tm_tiles[qt_i][:],
                        OP.mult,
                        OP.mult,
                    )
                    e16 = work_p.tile([P, NK], F16, tag="e16")
                    nc.scalar.activation(e16[:], u[:], AF.Exp)
                    em16 = em_p.tile([P, NK], F16, tag="em16")
                    zi = z_p.tile([P, 1], F32, tag="z")
                    nc.vector.scalar_tensor_tensor(
                        out=em16[:],
                        in0=e16[:],
                        scalar=1.0,
                        in1=cm_tiles[qt_i][:],
                        op0=OP.mult,
                        op1=OP.subtract,
                        accum_out=zi[:],
                    )
                    zinv_i = z_p.tile([P, 1], F32, tag="zinv")
                    nc.vector.reciprocal(zinv_i[:], zi[:])

                    last_bh_i = b == B_loc - 1 and h == H - 1
                    pgrp = 2
                    if qt_i % pgrp == 0:
                        p_sb = pst_p.tile([P, 2, NK], F32, tag="p")
                    spread = (b == B_loc - 1 and h == H - 1) or h == 0
                    if spread and qt_i % 2 == 1:
                        nc.vector.tensor_scalar(
                            p_sb[:, qt_i % pgrp], em16[:], zinv_i[:], None, OP.mult
                        )
                    else:
                        nc.gpsimd.tensor_scalar(
                            p_sb[:, qt_i % pgrp], em16[:], zinv_i[:], None, OP.mult
                        )
                    if qt_i % pgrp == pgrp - 1:
                        nc.sync.dma_start(
                            p_attn[
                                b, h, (qt_i - pgrp + 1) * P : (qt_i + 1) * P, :
                            ].rearrange("(so sp) k -> sp so k", sp=P),
                            p_sb[:, : pgrp],
                        )
                    emt_ps = psum_b.tile([P, KC * P], F16, tag="bank2k")
                    for kc in range(KC):
                        nc.tensor.transpose(
                            emt_ps[:, kc * P : (kc + 1) * P],
                            em16[:, kc * P : (kc + 1) * P],
                            ident16[:],
                        )
                    emt = emt_p.tile([P, KC, P], F16, tag="emt_sb")
                    nc.scalar.copy(emt[:], emt_ps[:])
                    o_ps = psum_o.tile([P, D], F32, tag="o_ps")
                    for kc in range(KC):
                        nc.tensor.matmul(
                            o_ps[:],
                            emt[:, kc],
                            v16[:, kc],
                            start=(kc == 0),
                            stop=(kc == KC - 1),
                        )
                    nc.scalar.activation(
                        o_sb[:, qt_i],
                        o_ps[:],
                        AF.Copy,
                        scale=zinv_i[:],
                    )
                out_r = out[b, h].rearrange("(so sp) d -> sp so d", sp=P)
                if b == B_loc - 1 and h == H - 1:
                    nc.sync.dma_start(out_r[:, :4], o_sb[:, :4])
                    nc.sync.dma_start(out_r[:, 4:], o_sb[:, 4:])
                else:
                    nc.sync.dma_start(out_r[:], o_sb[:])

    _split_multiwait(nc)
    return nc


_RUNNER_CACHE = {}


def _get_nc():
    if "nc" not in _RUNNER_CACHE:
        _RUNNER_CACHE["nc"] = build_attention_nc()
    return _RUNNER_CACHE["nc"]


def _get_runner():
    """Compile the NEFF-backed jitted executable once and reuse it.

    Mirrors concourse.bass2jax.run_bass_via_pjrt but without output-buffer
    donation, so the zero output operands stay device-resident and repeated
    kernel() calls skip both recompilation and zero re-upload.
    """
    if "runner" in _RUNNER_CACHE:
        return _RUNNER_CACHE["runner"]

    import jax
    import jax.numpy as jnp
    from jax.sharding import Mesh, PartitionSpec
    from jax.experimental.shard_map import shard_map

    from concourse.bass2jax import (
        _bass_exec_p,
        install_neuronx_cc_hook,
        partition_id_tensor,
    )

    nc = _get_nc()
    install_neuronx_cc_hook()
    partition_name = nc.partition_id_tensor.name if nc.partition_id_tensor else None

    in_names, out_names, out_avals, zero_shapes = [], [], [], []
    for alloc in nc.m.functions[0].allocations:
        if not isinstance(alloc, mybir.MemoryLocationSet):
            continue
        name = alloc.memorylocations[0].name
        if alloc.kind == "ExternalInput":
            if name != partition_name:
                in_names.append(name)
        elif alloc.kind == "ExternalOutput":
            out_names.append(name)
            shape = tuple(alloc.tensor_shape)
            dtype = mybir.dt.np(alloc.dtype)
            out_avals.append(jax.core.ShapedArray(shape, dtype))
            zero_shapes.append((shape, dtype))
    n_params = len(in_names)
    all_in_names = list(in_names) + list(out_names)
    if partition_name is not None:
        all_in_names.append(partition_name)

    def _body(*args):
        operands = list(args)
        if partition_name is not None:
            operands.append(partition_id_tensor())
        return tuple(
            _bass_exec_p.bind(
                *operands,
                out_avals=tuple(out_avals),
                in_names=tuple(all_in_names),
                out_names=tuple(out_names),
                lowering_input_output_aliases=(),
                sim_require_finite=True,
                sim_require_nnan=True,
                nc=nc,
            )
        )

    devices = [d for d in jax.devices() if d.platform != "cpu"]
    if len(devices) < N_CORES:
        devices = [d for d in jax.devices("axon") if d.platform != "cpu"]
    devices = devices[:N_CORES]
    assert len(devices) == N_CORES, (
        f"need {N_CORES} NeuronCores, have {len(devices)}"
    )
    mesh = Mesh(np.asarray(devices), ("core",))
    n_outs = len(out_names)
    fn = jax.jit(
        shard_map(
            _body,
            mesh=mesh,
            in_specs=(PartitionSpec("core"),) * (n_params + n_outs),
            out_specs=(PartitionSpec("core"),) * n_outs,
            check_rep=False,
        ),
        keep_unused=True,
    )
    zeros_dev = [
        jax.device_put(np.zeros((N_CORES * s[0], *s[1:]), d))
        for (s, d) in zero_shapes
    ]
    _RUNNER_CACHE["runner"] = (fn, in_names, out_names, zeros_dev)
    return _RUNNER_CACHE["runner"]


def _make_in_maps(query, key, value, time, mask):
    query = np.ascontiguousarray(query, dtype=np.float32)
    key = np.ascontiguousarray(key, dtype=np.float32)
    value = np.ascontiguousarray(value, dtype=np.float32)
    time = np.ascontiguousarray(time, dtype=np.float32)
    mask = np.ascontiguousarray(mask, dtype=np.int32)
    in_maps = []
    for c in range(N_CORES):
        sl = slice(c * B_LOC, (c + 1) * B_LOC)
        in_maps.append(
            {
                "q": np.ascontiguousarray(query[sl]),
                "k": np.ascontiguousarray(key[sl]),
                "v": np.ascontiguousarray(value[sl]),
                "time": np.ascontiguousarray(time[sl]),
                "mask": np.ascontiguousarray(mask[sl]),
            }
        )
    return in_maps


def _run_full(query, key, value, time, mask):
    fn, in_names, out_names, zeros_dev = _get_runner()
    full = {
        "q": np.ascontiguousarray(query, dtype=np.float32),
        "k": np.ascontiguousarray(key, dtype=np.float32),
        "v": np.ascontiguousarray(value, dtype=np.float32),
        "time": np.ascontiguousarray(time, dtype=np.float32),
        "mask": np.ascontiguousarray(mask, dtype=np.int32),
    }
    ins = [full[nm] for nm in in_names]
    outs = fn(*ins, *zeros_dev)
    res = {nm: np.asarray(o) for nm, o in zip(out_names, outs)}
    return res


def kernel(query, key, value, time, mask):
    """Full-input entry point: shards over 8 NeuronCores, returns
    (out, p_attn) matching the reference."""
    res = _run_full(query, key, value, time, mask)
    return res["out"], res["p_attn"]


def _run(in_maps, **kwargs):
    # debug/profiling path through the stock SPMD driver
    nc = _get_nc()
    return run_bass_kernel_spmd(nc, in_maps, core_ids=list(range(N_CORES)), **kwargs)
